# revision 18
# baseline (speedup 1.0000x reference)
"""ClusterAttention Trainium2 kernel (fp8/bf16, software-pipelined).

Per cluster k (256 clusters, 256 points, dim 512, 8 heads):
    qkv = feat @ qkv_w; attn = softmax(scale*q@k^T + pos_bias + mask_bias)
    out = (attn @ v) @ proj_w

Sharding: pure data parallel over clusters across 8 NeuronCores (32
clusters/core); small weights replicated.

Key implementation choices (all exact or within tolerance under softmax):
  - pos_bias[a,b,h] = P[b,h] - P[a,h]; the -P[a,h] term and pos_b are
    constant along the key axis b and cancel in softmax.  The remaining
    per-key bias is folded in MULTIPLICATIVELY: exp(s + bias_b) =
    exp(s)*exp(bias_b), with ebias = exp(P + 100*(mask-1)) precomputed on
    the HOST and multiplied into v (and into the ones-columns used for
    the softmax denominator).  Masked keys get ebias ~ e^-100 -> 0 in
    bf16, i.e. exact masking.  This removes the per-head bias from the
    Exp activation.
  - q/k projection runs in fp8e4m3 DoubleRow perf mode (2 contraction
    subtiles per instruction, 0.5 cyc/row); weights are host-prescaled by
    64 to stay in fp8's normal range, compensated in the Exp scale.
    v/S^T/attn@v/proj run in bf16 (1 cyc/row).  PSUM accumulation fp32.
    Measured end-to-end rel err ~1.4e-2 vs 2e-2 tolerance.
  - feat arrives HOST-pretransposed ([kc, 4, 128, 256] channel-major) so
    no PE transposes are needed for q/k/v; loads batched 8 clusters/DMA.
  - Softmax denominator via ebias-valued columns appended to v (cols
    64:66); normalization is a per-partition reciprocal+multiply.
  - Two-stage software pipeline: cluster k's front half (qk, v, S^T+exp)
    is issued before cluster k-1's back half (attn@v, norm, x^T, proj) so
    each engine's in-order queue always holds independent work while the
    Act engine drains the 8 Exp instructions of the previous cluster.
"""

import numpy as np

NCORES = 8
KC_TOTAL, M, DIM = 256, 256, 512
H, HD = 8, 64
KC = KC_TOTAL // NCORES  # clusters per core
SCALE = HD ** -0.5
G = 8  # clusters per feat DMA batch
FP8_ST = True  # fp8 DoubleRow for S^T (q/k stored fp8, head-repacked)

_cache = {}


def _build_program():
    import concourse.bass as bass
    import concourse.tile as tile
    from concourse import bacc, mybir
    from concourse.masks import make_identity

    f32 = mybir.dt.float32
    bf16 = mybir.dt.bfloat16
    f8 = mybir.dt.float8e4
    DR = mybir.MatmulPerfMode.DoubleRow
    Exp = mybir.ActivationFunctionType.Exp

    nc = bacc.Bacc("TRN2", target_bir_lowering=False, debug=False,
                   num_devices=NCORES)

    featT_d = nc.dram_tensor("featT", [KC, 4, 128, M], bf16,
                             kind="ExternalInput").ap()
    featT8_d = nc.dram_tensor("featT8", [KC, 4, 128, M], f8,
                              kind="ExternalInput").ap()
    ebias_d = nc.dram_tensor("ebias", [KC, 2, 128, H], bf16,
                             kind="ExternalInput").ap()
    wqk_d = nc.dram_tensor("wqk8", [4, 128, 2 * DIM], f8,
                           kind="ExternalInput").ap()
    wv_d = nc.dram_tensor("wv", [4, 128, DIM], bf16,
                          kind="ExternalInput").ap()
    wproj_d = nc.dram_tensor("wproj", [4, 128, DIM], bf16,
                             kind="ExternalInput").ap()
    out_d = nc.dram_tensor("out", [KC, M, DIM], f32, kind="ExternalOutput").ap()

    with tile.TileContext(nc) as tc:
        import contextlib
        ctx = contextlib.ExitStack()
        with ctx:
            wp = ctx.enter_context(tc.tile_pool(name="weights", bufs=1))
            featp = ctx.enter_context(tc.tile_pool(name="featp", bufs=2))
            qkTp = ctx.enter_context(tc.tile_pool(name="qkTp", bufs=3))
            vp = ctx.enter_context(tc.tile_pool(name="vp", bufs=3))
            expp = ctx.enter_context(tc.tile_pool(name="expp", bufs=3))
            smallp = ctx.enter_context(tc.tile_pool(name="smallp", bufs=4))
            xp = ctx.enter_context(tc.tile_pool(name="xp", bufs=3))
            xTp = ctx.enter_context(tc.tile_pool(name="xTp", bufs=3))
            outp = ctx.enter_context(tc.tile_pool(name="outp", bufs=3))

            qkv_ps = ctx.enter_context(tc.tile_pool(name="qkv_ps", bufs=2, space="PSUM"))
            st_ps = ctx.enter_context(tc.tile_pool(name="st_ps", bufs=2, space="PSUM"))
            o_ps = ctx.enter_context(tc.tile_pool(name="o_ps", bufs=2, space="PSUM"))
            qk_ps = vp_ps = qkv_ps

            # ---- persistent weights / per-core constants in SBUF ----
            wqk_sb = wp.tile([128, 4, 2 * DIM], f8)
            nc.sync.dma_start(out=wqk_sb, in_=wqk_d.rearrange("kt p n -> p kt n"))
            wv_sb = wp.tile([128, 4, DIM], bf16)
            nc.sync.dma_start(out=wv_sb, in_=wv_d.rearrange("kt p n -> p kt n"))
            wproj_sb = wp.tile([128, 4, DIM], bf16)
            nc.sync.dma_start(out=wproj_sb, in_=wproj_d.rearrange("kt p n -> p kt n"))
            ebias_all = wp.tile([128, KC, 2, H], bf16)
            nc.sync.dma_start(out=ebias_all,
                              in_=ebias_d.rearrange("kc bt p h -> p kc bt h"))
            ident = wp.tile([128, 128], bf16)
            make_identity(nc, ident)

            state = {}  # per-cluster tiles passed from phase1 to phase2
            featbig = {}

            def load_qk_v(kk):
                """loads, q/k projection (fp8 DoubleRow), v+ebias."""
                if kk % G == 0:
                    fb = featp.tile([128, G, 4, M], bf16)
                    nc.sync.dma_start(
                        out=fb,
                        in_=featT_d[kk:kk + G].rearrange("g ct p m -> p g ct m"))
                    fb8 = featp.tile([128, G, 4, M], f8)
                    nc.sync.dma_start(
                        out=fb8,
                        in_=featT8_d[kk:kk + G].rearrange("g ct p m -> p g ct m"))
                    featbig["bf"] = fb
                    featbig["f8"] = fb8
                featT = featbig["bf"][:, kk % G]
                featT8 = featbig["f8"][:, kk % G]
                ebias_sb = ebias_all[:, kk]

                # q^T,k^T: qkT[n%128, nt, z, m]; z=1 slots are persistent
                # zeros (DoubleRow padding for the hd=64 contraction of S^T).
                qkT = qkTp.tile([128, 8, 2, M], f8)
                if kk < 3:  # zero the pad slots once per pool buffer
                    nc.vector.memset(qkT[:, :, 1, :], 0.0)
                for g in range(4):
                    ps = qk_ps.tile([128, 2, M], f32, tag="qk")
                    for sub in range(2):
                        nt = 2 * g + sub
                        for i in range(2):
                            nc.tensor.matmul(
                                ps[:, sub],
                                lhsT=wqk_sb[:, 2 * i:2 * i + 2,
                                            nt * 128:(nt + 1) * 128],
                                rhs=featT8[:, 2 * i:2 * i + 2, :],
                                start=(i == 0), stop=(i == 1),
                                perf_mode=DR)
                    if g == 0:
                        nc.scalar.copy(out=qkT[:, 2 * g:2 * g + 2, 0, :], in_=ps)
                    else:
                        nc.vector.tensor_copy(out=qkT[:, 2 * g:2 * g + 2, 0, :],
                                              in_=ps)

                # v (natural), ebias folded in; cols 64:66 = ebias
                vaug = vp.tile([128, 2, H, HD + 2], bf16)
                nc.gpsimd.tensor_copy(
                    out=vaug[:, :, :, HD:HD + 2],
                    in_=ebias_all[:, kk].broadcast_to([128, 2, H, 2]))
                for bt in range(2):
                    ps = vp_ps.tile([128, DIM], f32, tag="vp")
                    for kt in range(4):
                        nc.tensor.matmul(ps,
                                         lhsT=featT[:, kt, bt * 128:(bt + 1) * 128],
                                         rhs=wv_sb[:, kt, :],
                                         start=(kt == 0), stop=(kt == 3))
                    nc.vector.tensor_mul(
                        out=vaug[:, bt, :, 0:HD],
                        in0=ps.rearrange("p (h d) -> p h d", h=H),
                        in1=ebias_sb[:, bt, :].broadcast_to([128, H, HD]))
                state[kk] = {"qkT": qkT, "vaug": vaug}

            def st_exp_head(kk, h):
                """S^T + exp for one head (one 1-bank psum tile)."""
                s = state[kk]
                qkT = s["qkT"]
                if h == 0:
                    s["expst"] = expp.tile([128, H, 2, M], bf16, name="expst")
                expst = s["expst"]
                st = st_ps.tile([128, 2, M], f32, tag="st")
                ro = (h % 2) * 64
                for bt in range(2):
                    nc.tensor.matmul(
                        st[:, bt],
                        lhsT=qkT[ro:ro + 64, 4 + h // 2, :,
                                 bt * 128:(bt + 1) * 128],
                        rhs=qkT[ro:ro + 64, h // 2, :, :],
                        start=True, stop=True, perf_mode=DR)
                nc.scalar.activation(out=expst[:, h], in_=st, func=Exp,
                                     scale=SCALE / 4096.0)

            def attn_group(kk, at, hg):
                """attn@v + normalize for one (query-block, head-group)."""
                s = state[kk]
                expst, vaug = s["expst"], s["vaug"]
                if "x" not in s:
                    s["x"] = xp.tile([128, 2, DIM], bf16, name="x_sb")
                    s["recips"] = smallp.tile([128, 2, 2, 4], f32, name="recips")
                x_sb, recips = s["x"], s["recips"]
                ops = o_ps.tile([128, 4, HD + 2], f32, tag="o")
                for hh in range(4):
                    h = hg * 4 + hh
                    for bt in range(2):
                        nc.tensor.matmul(
                            ops[:, hh],
                            lhsT=expst[:, h, bt, at * 128:(at + 1) * 128],
                            rhs=vaug[:, bt, h, :],
                            start=(bt == 0), stop=(bt == 1))
                nc.vector.reciprocal(out=recips[:, at, hg, :], in_=ops[:, :, HD])
                nc.vector.tensor_mul(
                    out=x_sb[:, at, hg * 256:(hg + 1) * 256].rearrange(
                        "p (h d) -> p h d", h=4),
                    in0=ops[:, :, 0:HD],
                    in1=recips[:, at, hg, :].broadcast_to([128, 4, HD]))

            def transpose_x(kk, ct):
                """x^T for one 128-column block."""
                s = state[kk]
                if "xT" not in s:
                    s["xT"] = xTp.tile([128, 4, M], bf16, name="xT")
                x_sb, xT = s["x"], s["xT"]
                tp = o_ps.tile([128, 256], bf16, tag="o")
                for at in range(2):
                    nc.tensor.transpose(tp[:, at * 128:(at + 1) * 128],
                                        x_sb[:, at, ct * 128:(ct + 1) * 128],
                                        ident)
                nc.vector.tensor_copy(out=xT[:, ct, :], in_=tp)

            def proj_store(kk):
                """final projection and store."""
                s = state.pop(kk)
                xT = s["xT"]
                out_sb = outp.tile([128, 2, DIM], f32)
                for at in range(2):
                    ps = vp_ps.tile([128, DIM], f32, tag="vp")
                    for kt in range(4):
                        nc.tensor.matmul(ps,
                                         lhsT=xT[:, kt, at * 128:(at + 1) * 128],
                                         rhs=wproj_sb[:, kt, :],
                                         start=(kt == 0), stop=(kt == 3))
                    nc.scalar.copy(out=out_sb[:, at], in_=ps)
                nc.sync.dma_start(
                    out=out_d[kk].rearrange("(t p) c -> p t c", p=128),
                    in_=out_sb)

            groups = [(0, 0), (1, 0), (0, 1), (1, 1)]
            for kk in range(KC + 1):
                if kk < KC:
                    load_qk_v(kk)
                for j in range(4):
                    if kk < KC:
                        st_exp_head(kk, 2 * j)
                        st_exp_head(kk, 2 * j + 1)
                    if kk >= 1:
                        at, hg = groups[j]
                        attn_group(kk - 1, at, hg)
                        if j == 2:
                            transpose_x(kk - 1, 0)
                            transpose_x(kk - 1, 1)
                        if j == 3:
                            transpose_x(kk - 1, 2)
                            transpose_x(kk - 1, 3)
                if kk >= 1:
                    proj_store(kk - 1)

    nc.compile()
    return nc


def get_program():
    if "nc" not in _cache:
        _cache["nc"] = _build_program()
    return _cache["nc"]


def make_in_maps(pos, feat, qkv_w, qkv_b, pos_w, proj_w, mask):
    """Host-side prep: pretranspose feat, precompute exp-bias, shard."""
    import ml_dtypes
    bf16 = ml_dtypes.bfloat16
    f8 = ml_dtypes.float8_e4m3

    pos = np.asarray(pos, dtype=np.float32)
    feat = np.asarray(feat, dtype=np.float32)
    mask = np.asarray(mask, dtype=np.int32)
    qkv_w = np.asarray(qkv_w, dtype=np.float32)
    proj_w = np.asarray(proj_w, dtype=np.float32)
    pos_w = np.asarray(pos_w, dtype=np.float32)

    # featT[k, ct, p, m] = feat[k, m, ct*128+p]
    featTf = feat.transpose(0, 2, 1).reshape(KC_TOTAL, 4, 128, M)
    featT = np.ascontiguousarray(featTf.astype(bf16))
    featT8 = np.ascontiguousarray(featTf.astype(f8))
    # ebias[k, b, h] = exp(pos_n@pos_w + 100*(mask-1)); masked -> 0 in bf16
    pos_n = pos / pos.max(axis=(0, 1), keepdims=True)
    P = pos_n @ pos_w  # [k, m, H]
    eb = np.exp(P + 100.0 * (mask.astype(np.float32) - 1.0))
    ebias = np.ascontiguousarray(
        eb.reshape(KC_TOTAL, 2, 128, H).astype(bf16))

    wqk8 = np.ascontiguousarray(
        (qkv_w[:, :2 * DIM] * 64.0).reshape(4, 128, 2 * DIM).astype(f8))
    wv = np.ascontiguousarray(
        qkv_w[:, 2 * DIM:].reshape(4, 128, DIM).astype(bf16))
    wproj = np.ascontiguousarray(proj_w.reshape(4, 128, DIM).astype(bf16))

    in_maps = []
    for i in range(NCORES):
        sl = slice(i * KC, (i + 1) * KC)
        in_maps.append({
            "featT": featT[sl], "featT8": featT8[sl], "ebias": ebias[sl],
            "wqk8": wqk8, "wv": wv, "wproj": wproj,
        })
    return in_maps


def kernel(pos, feat, qkv_w, qkv_b, pos_w, pos_b, proj_w, proj_b, mask):
    from concourse.bass_utils import run_bass_kernel_spmd

    # Structurally zero in this problem's setup; the device program relies
    # on it (v-channel/proj biases; pos_b cancels in softmax).
    assert np.abs(np.asarray(qkv_b)).max() == 0.0
    assert np.abs(np.asarray(proj_b)).max() == 0.0

    nc = get_program()
    in_maps = make_in_maps(pos, feat, qkv_w, qkv_b, pos_w, proj_w, mask)
    res = run_bass_kernel_spmd(nc, in_maps, list(range(NCORES)))
    out = np.concatenate([res.results[i]["out"] for i in range(NCORES)], axis=0)
    return out.astype(np.float32)
